# revision 7
# baseline (speedup 1.0000x reference)
# GAT layer on 8 NeuronCores — optimized v2.
#
# Same target-sharded strategy as the baseline kernel (each core owns 1/8 of
# the target nodes and the edges into them; one scalar AllReduce-max for the
# reference's softmax epsilon), restructured to cut per-instruction overheads:
#   - Gather-table rows are [proj bf16 x128 | s_src f32 x4 | pad] (512B): the
#     s_src hi/lo bf16 split is gone (f32 rides in the row), saving DVE ops.
#   - Phase T streams x in groups of 8 tiles per DMA and writes table rows in
#     one DMA per group (HWDGE descriptor-gen is ~625ns per DMA instruction —
#     instruction count matters more than bytes).
#   - PSUM->SBUF copies are split between the Activation and DVE engines.
#   - The local s_trg table, the per-window aggregates, and the local x tiles
#     stay resident in SBUF (no DRAM round trips).
#   - Per-window tile counts are ragged (per-window max edge count over cores
#     instead of global max), trimming gather/compute padding.
#   - Phase E per-window inputs (gather idxs + rel) are packed in one tensor;
#     window pairs share one DMA. Phase F outputs are stored 4 windows/DMA.
import sys
from contextlib import ExitStack

import numpy as np

sys.path.insert(0, "/opt/trn_rl_repo")

import ml_dtypes  # noqa: E402

import concourse.bass as bass  # noqa: E402,F401
import concourse.mybir as mybir  # noqa: E402
import concourse.tile as tile  # noqa: E402
from concourse import bacc  # noqa: E402
from concourse.masks import make_identity  # noqa: E402

P = 128
NH, FOUT = 4, 32
NHF = NH * FOUT  # 128
FIN = 128
ROW = 256  # bf16 elems per gather-table row (512B)
LEAKY = 0.2
SHIFT = 24.0
F32 = mybir.dt.float32
BF16 = mybir.dt.bfloat16
FP8 = mybir.dt.float8e4
I16 = mybir.dt.int16
AX = mybir.AxisListType
OP = mybir.AluOpType
ACT = mybir.ActivationFunctionType
BF = ml_dtypes.bfloat16
F8 = ml_dtypes.float8_e4m3
GT = 8  # x tiles per phase-T DMA group


def _prepare_edges(edge_index, n_nodes, n_cores):
    """Host-side integer prep: per-core packed gather indices + rel cols +
    transposed one-hot (selt). Per-window ragged tile counts (max over cores).

    Returns (t_as, t_bs, gi, selt):
      t_as/t_bs: [nw] int arrays of A/B-half tile counts per window
      gi:   [n_cores, 128, sum_w(t_w*9)] int16 — per window: t_aw*8 cols of
            A idxs, t_bw*8 cols of B idxs, then t_w cols of rel (bf16 bits)
      selt: [n_cores, 128, sum_w(t_w*128)] bf16 — transposed one-hot
            (partition = target rel, col = edge slot)
    """
    npc = n_nodes // n_cores
    nw = (npc + P - 1) // P
    half = n_nodes // 2
    src = np.ascontiguousarray(edge_index[0]).astype(np.int64)
    trg = np.ascontiguousarray(edge_index[1]).astype(np.int64)
    E = src.shape[0]
    c = trg // npc
    tl = trg % npc
    w = tl // P
    rel = tl % P
    isb = (src >= half).astype(np.int64)
    key = (c * nw + w) * 2 + isb
    cnt = np.bincount(key, minlength=n_cores * nw * 2)
    cnt_a = cnt[0::2].reshape(n_cores, nw)
    cnt_b = cnt[1::2].reshape(n_cores, nw)
    t_as = np.ceil(cnt_a.max(axis=0) / P).astype(np.int64)
    t_bs = np.ceil(cnt_b.max(axis=0) / P).astype(np.int64)
    t_ws = t_as + t_bs

    order = np.argsort(key, kind="stable")
    key_s = key[order]
    src_s, c_s, w_s, rel_s, isb_s = (src[order], c[order], w[order],
                                     rel[order], isb[order])
    gstart = np.concatenate([[0], np.cumsum(cnt)])[:-1]
    jj = np.arange(E) - gstart[key_s]
    t_loc = jj // P
    p_idx = jj % P
    t_idx = t_loc + np.where(isb_s == 1, t_as[w_s], 0)

    # column offsets per window
    ioff = np.concatenate([[0], np.cumsum(t_ws * 8)])   # idx cols
    roff = np.concatenate([[0], np.cumsum(t_ws)])       # rel cols
    soff = np.concatenate([[0], np.cumsum(t_ws * P)])   # selt cols
    icols = int(ioff[-1])
    rcols = int(roff[-1])
    scols = int(soff[-1])

    # gather idx block: flat pos k = t_idx*128 + p_idx -> row k%16, col k//16
    k = t_idx * P + p_idx
    gidx16 = np.zeros((n_cores, 16, icols), np.int16)
    gidx16[c_s, k % 16, ioff[w_s] + k // 16] = \
        (src_s - isb_s * half).astype(np.int16)
    gidx = np.tile(gidx16, (1, 8, 1))  # [n_cores, 128, icols]

    relarr = np.full((n_cores, P, rcols), -1.0, BF)
    relarr[c_s, p_idx, roff[w_s] + t_idx] = rel_s.astype(BF)

    selt = np.zeros((n_cores, P, scols), F8)
    selt[c_s, rel_s, soff[w_s] + t_idx * P + p_idx] = 1.0
    sele = np.zeros((n_cores, P, scols), F8)
    sele[c_s, p_idx, soff[w_s] + t_idx * P + rel_s] = 1.0
    # exact idx counts per (window, half): gathers can skip trailing padding
    cnt_am = cnt_a.max(axis=0)
    cnt_bm = cnt_b.max(axis=0)
    n_as = (np.ceil(cnt_am / 16) * 16).astype(np.int64)
    n_bs = (np.ceil(cnt_bm / 16) * 16).astype(np.int64)

    # interleave per window: idx block then rel block (as int16 bits)
    gi = np.zeros((n_cores, P, icols + rcols), np.int16)
    goff = 0
    for wi in range(nw):
        tw = int(t_ws[wi])
        gi[:, :, goff:goff + tw * 8] = gidx[:, :, ioff[wi]:ioff[wi + 1]]
        gi[:, :, goff + tw * 8:goff + tw * 9] = \
            relarr[:, :, roff[wi]:roff[wi + 1]].view(np.int16)
        goff += tw * 9
    return t_as, t_bs, n_as, n_bs, gi, selt, sele


def build_bass(n_nodes, n_cores, t_as, t_bs, n_as=None, n_bs=None):
    npc = n_nodes // n_cores
    nw = (npc + P - 1) // P
    nt = (n_nodes + P - 1) // P
    half = n_nodes // 2
    t_ws = [int(a + b) for a, b in zip(t_as, t_bs)]
    if n_as is None:
        n_as = [int(t) * P for t in t_as]
    if n_bs is None:
        n_bs = [int(t) * P for t in t_bs]
    t_max = max(t_ws)
    gicols = sum(t * 9 for t in t_ws)
    scols = sum(t * P for t in t_ws)
    nc = bacc.Bacc("TRN2", target_bir_lowering=False, debug=False,
                   num_devices=n_cores)

    x = nc.dram_tensor("x", [n_nodes, FIN], F32, kind="ExternalInput")
    xloc = nc.dram_tensor("xloc", [npc, FIN], F32, kind="ExternalInput")
    w_in = nc.dram_tensor("W", [FIN, NHF], F32, kind="ExternalInput")
    amat = nc.dram_tensor("amat", [NHF, 2 * NH], F32, kind="ExternalInput")
    bias_in = nc.dram_tensor("bias", [1, NHF], F32, kind="ExternalInput")
    gi_in = nc.dram_tensor("gi", [P, gicols], I16, kind="ExternalInput")
    selt_in = nc.dram_tensor("selt", [P, scols], FP8, kind="ExternalInput")
    sele_in = nc.dram_tensor("sele", [P, scols], FP8, kind="ExternalInput")
    out = nc.dram_tensor("out", [npc, NHF], F32, kind="ExternalOutput")

    ntab = nt * P
    tab = nc.dram_tensor("tab", [ntab, ROW], BF16)

    with tile.TileContext(nc) as tc, ExitStack() as ctx:
        const = ctx.enter_context(tc.tile_pool(name="const", bufs=1))
        sb = ctx.enter_context(tc.tile_pool(name="sb", bufs=3))
        sbg = ctx.enter_context(tc.tile_pool(name="sbg", bufs=2))
        dram = ctx.enter_context(tc.tile_pool(name="dram", bufs=1, space="DRAM"))

        ident = const.tile([P, P], F32)
        make_identity(nc, ident[:])
        c_i32 = const.tile([P, P], mybir.dt.int32)
        nc.gpsimd.iota(c_i32[:], pattern=[[1, P]], base=0, channel_multiplier=0)
        c_bf = const.tile([P, P], BF16)
        nc.vector.tensor_copy(c_bf[:], c_i32[:])
        ident_bf = const.tile([P, P], BF16)
        nc.vector.tensor_copy(ident_bf[:], ident[:])

        sb_w = const.tile([FIN, NHF], F32)
        nc.sync.dma_start(sb_w[:], w_in[:])
        sb_a = const.tile([NHF, 2 * NH], F32)
        nc.sync.dma_start(sb_a[:], amat[:])
        sb_bias = const.tile([1, NHF], F32)
        nc.sync.dma_start(sb_bias[:], bias_in[:])

        with tc.tile_pool(name="ps0", bufs=1, space="PSUM") as ps0:
            ps_wt = ps0.tile([NHF, FIN], F32, tag="pst")
            nc.tensor.transpose(ps_wt[:], sb_w[:], ident[:])
            sb_wt = sb.tile([NHF, FIN], F32)
            nc.vector.tensor_copy(sb_wt[:], ps_wt[:])
            ps_wa = ps0.tile([FIN, 2 * NH], F32, tag="pst2")
            nc.tensor.matmul(ps_wa[:], lhsT=sb_wt[:], rhs=sb_a[:], start=True,
                             stop=True)
            wcat = const.tile([FIN, NHF + 2 * NH], F32)
            nc.vector.tensor_copy(wcat[:, 0:NHF], sb_w[:])
            nc.vector.tensor_copy(wcat[:, NHF:NHF + 2 * NH], ps_wa[:])
            ones_row = const.tile([1, P], F32)
            nc.gpsimd.memset(ones_row[:], 1.0)
            ps_b = ps0.tile([P, NHF], F32, tag="pst3")
            nc.tensor.matmul(ps_b[:], lhsT=ones_row[:], rhs=sb_bias[:],
                             start=True, stop=True)
            sb_b = const.tile([P, NHF], F32)
            nc.vector.tensor_copy(sb_b[:], ps_b[:])

        bias_zero = const.tile([P, 1], F32)
        nc.gpsimd.memset(bias_zero[:], 0.0)
        bias_mshift = const.tile([P, 1], F32)
        nc.gpsimd.memset(bias_mshift[:], -SHIFT)
        bias_meps = const.tile([P, 1], F32)
        nc.gpsimd.memset(bias_meps[:], float(-SHIFT + np.log(1e-16)))

        # resident tiles
        stw_bf = const.tile([P, nw * 2 * NH], BF16)   # local s_trg hi|lo
        xloc_sb = const.tile([P, nw * FIN], F32)      # local x (phase F skip)
        acc_wt = const.tile([P, nw * P], F32)         # weighted sums (featT)
        acc_d = const.tile([NH, nw * P], F32)         # denominators

        psT_cm = tc.tile_pool(name="psT", bufs=3, space="PSUM")
        psT = psT_cm.__enter__()

        # --- mini-pass: local s_trg table + resident xloc tiles ---
        for g in range(0, nw, GT):
            gw = min(GT, nw - g)
            r0 = g * P
            rows = min(gw * P, npc - r0)
            full = rows // P
            rem = rows - full * P
            xt = xloc_sb[:, r0:r0 + gw * FIN].rearrange(
                "p (c f) -> p c f", f=FIN)
            if full > 0:
                nc.sync.dma_start(
                    xt[:, 0:full, :],
                    xloc[r0:r0 + full * P, :].rearrange(
                        "(c p) f -> p c f", p=P))
            if rem > 0:
                nc.gpsimd.memset(xt[:, full, :], 0.0)
                nc.sync.dma_start(xt[:rem, full, :],
                                  xloc[r0 + full * P:r0 + rows, :])
            for ci in range(gw):
                rc = min(P, npc - (g + ci) * P)
                ps_xt = psT.tile([P, P], F32, tag="ps_xt")
                nc.tensor.transpose(ps_xt[:, :rc], xt[:rc, ci, :],
                                    ident[:rc, :rc])
                x_tr = sb.tile([P, P], F32, tag="x_tr")
                nc.scalar.copy(x_tr[:, :rc], ps_xt[:, :rc])
                ps_c = psT.tile([P, NH], F32, tag="ps_tab")
                nc.tensor.matmul(ps_c[:rc, :], lhsT=x_tr[:, :rc],
                                 rhs=wcat[:, NHF + NH:NHF + 2 * NH],
                                 start=True, stop=True)
                wi = g + ci
                stw = stw_bf[:, wi * 2 * NH:(wi + 1) * 2 * NH]
                if rc < P:
                    nc.gpsimd.memset(stw, 0.0)
                nc.vector.tensor_copy(stw[:rc, 0:NH], ps_c[:rc, :])
                c_lo = sb.tile([P, NH], F32, tag="c_lo")
                nc.vector.tensor_tensor(c_lo[:rc], ps_c[:rc, :],
                                        stw[:rc, 0:NH], OP.subtract)
                nc.gpsimd.tensor_copy(stw[:rc, NH:2 * NH], c_lo[:rc])

        # --- phase T: gather table for all nodes ---
        for g in range(0, nt, GT):
            gw = min(GT, nt - g)
            r0 = g * P
            rows = min(gw * P, n_nodes - r0)
            full = rows // P
            rem = rows - full * P
            xt8 = sb.tile([P, GT * FIN], F32, tag="xt8")
            x3 = xt8[:, 0:gw * FIN].rearrange("p (c f) -> p c f", f=FIN)
            if full > 0:
                nc.sync.dma_start(
                    x3[:, 0:full, :],
                    x[r0:r0 + full * P, :].rearrange("(c p) f -> p c f", p=P))
            if rem > 0:
                nc.sync.dma_start(x3[:rem, full, :],
                                  x[r0 + full * P:r0 + rows, :])
            tabt = sb.tile([P, GT * ROW], BF16, tag="tabt")
            t3 = tabt[:, 0:gw * ROW].rearrange("p (c r) -> p c r", r=ROW)
            t3f = tabt[:, 0:gw * ROW].bitcast(F32).rearrange(
                "p (c r) -> p c r", r=ROW // 2)
            NSS = NHF + NH
            for c0 in range(0, gw, 2):
                cn = min(2, gw - c0)
                ps_xt = psT.tile([P, 2 * P], F32, tag="ps_xt")
                for ci in range(c0, c0 + cn):
                    rc = min(P, n_nodes - (g + ci) * P)
                    nc.tensor.transpose(
                        ps_xt[:, (ci - c0) * P:(ci - c0) * P + rc],
                        x3[:rc, ci, :], ident[:rc, :rc])
                x_tr = sb.tile([P, 2 * P], F32, tag="x_tr")
                eng0 = (nc.scalar.copy, nc.vector.tensor_copy)[(c0 // 2) % 2]
                eng1 = (nc.vector.tensor_copy, nc.scalar.copy)[(c0 // 2) % 2]
                rc_last = min(P, n_nodes - (g + c0 + cn - 1) * P)
                eng0(x_tr[:, 0:(cn - 1) * P + rc_last],
                     ps_xt[:, 0:(cn - 1) * P + rc_last])
                ps_tab = psT.tile([P, 2 * NSS], F32, tag="ps_tab")
                for ci in range(c0, c0 + cn):
                    rc = min(P, n_nodes - (g + ci) * P)
                    nc.tensor.matmul(
                        ps_tab[:rc, (ci - c0) * NSS:(ci - c0 + 1) * NSS],
                        lhsT=x_tr[:, (ci - c0) * P:(ci - c0) * P + rc],
                        rhs=wcat[:, 0:NSS], start=True, stop=True)
                p3 = ps_tab[:, 0:cn * NSS].rearrange("p (c s) -> p c s", s=NSS)
                if rc_last == P:
                    eng1(t3[:, c0:c0 + cn, 0:NHF], p3[:, :, 0:NHF])
                    eng1(t3f[:, c0:c0 + cn, 64:64 + NH],
                         p3[:, :, NHF:NHF + NH])
                else:
                    for ci in range(c0, c0 + cn):
                        rc = min(P, n_nodes - (g + ci) * P)
                        eng1(t3[:rc, ci, 0:NHF], p3[:rc, ci - c0, 0:NHF])
                        eng1(t3f[:rc, ci, 64:64 + NH],
                             p3[:rc, ci - c0, NHF:NHF + NH])
            NS = NHF + 2 * NH  # 136 slots: proj + s_src f32 (pad never read)
            if full > 0:
                nc.sync.dma_start(
                    tab[r0:r0 + full * P, 0:NS].rearrange(
                        "(c p) r -> p c r", p=P),
                    t3[:, 0:full, 0:NS])
            if rem > 0:
                nc.sync.dma_start(tab[r0 + full * P:r0 + rows, 0:NS],
                                  t3[:rem, full, 0:NS])

        psT_cm.__exit__(None, None, None)

        # --- phase E ---
        zmax = const.tile([P, t_max * NH], F32)
        nc.gpsimd.memset(zmax[:], -1e30)
        psE = ctx.enter_context(tc.tile_pool(name="psE", bufs=2, space="PSUM"))

        # window-pair shared input loads
        goffs = np.concatenate([[0], np.cumsum([t * 9 for t in t_ws])])
        soffs = np.concatenate([[0], np.cumsum([t * P for t in t_ws])])
        gi_t = {}
        selt_t = {}
        sele_t = {}
        for w0 in range(0, nw, 2):
            wn = min(2, nw - w0)
            gcols = int(goffs[w0 + wn] - goffs[w0])
            scols_p = int(soffs[w0 + wn] - soffs[w0])
            git = sb.tile([P, 2 * (t_max * 9)], I16, tag="git", bufs=3)
            nc.sync.dma_start(git[:, 0:gcols],
                              gi_in[:, int(goffs[w0]):int(goffs[w0]) + gcols])
            st = sbg.tile([P, 2 * (t_max * P)], FP8, tag="st", bufs=3)
            nc.sync.dma_start(st[:, 0:scols_p],
                              selt_in[:, int(soffs[w0]):int(soffs[w0]) + scols_p])
            se = sbg.tile([P, 2 * (t_max * P)], FP8, tag="se", bufs=3)
            nc.sync.dma_start(se[:, 0:scols_p],
                              sele_in[:, int(soffs[w0]):int(soffs[w0]) + scols_p])
            for wi in range(w0, w0 + wn):
                co = int(goffs[wi] - goffs[w0])
                so = int(soffs[wi] - soffs[w0])
                gi_t[wi] = git[:, co:co + t_ws[wi] * 9]
                selt_t[wi] = st[:, so:so + t_ws[wi] * P]
                sele_t[wi] = se[:, so:so + t_ws[wi] * P]

        for w in range(nw):
            t_a, t_b = int(t_as[w]), int(t_bs[w])
            t_w = t_a + t_b
            giw = gi_t[w]
            selt = selt_t[w]
            sel = sele_t[w]
            gath = sbg.tile([P, t_max * ROW], BF16, tag="gath", bufs=4)
            g3 = gath[:, 0:t_w * ROW].rearrange("p (t c) -> p t c", c=ROW)
            g3f = gath[:, 0:t_w * ROW].bitcast(F32).rearrange(
                "p (t c) -> p t c", c=ROW // 2)
            if t_a > 0:
                nc.gpsimd.dma_gather(
                    out_ap=g3[:, 0:t_a, :], in_ap=tab[0:half, :],
                    idxs_ap=giw[:, 0:t_a * 8], num_idxs=t_a * P,
                    num_idxs_reg=t_a * P, elem_size=ROW, single_packet=False)
            if t_b > 0:
                nc.gpsimd.dma_gather(
                    out_ap=g3[:, t_a:t_w, :], in_ap=tab[half:ntab, :],
                    idxs_ap=giw[:, t_a * 8:t_w * 8], num_idxs=t_b * P,
                    num_idxs_reg=t_b * P, elem_size=ROW, single_packet=False)

            # z = s_trg(hi)+s_trg(lo)+s_src accumulated in PSUM:
            # two one-hot matmuls + an identity matmul for the gathered s_src
            ps_st = psE.tile([P, t_max * NH], F32, tag="ps_st")
            for t in range(t_w):
                seg = ps_st[:, t * NH:(t + 1) * NH]
                nc.tensor.matmul(seg, lhsT=selt[:, t * P:(t + 1) * P],
                                 rhs=stw_bf[:, w * 2 * NH:w * 2 * NH + NH],
                                 start=True, stop=False)
                nc.tensor.matmul(seg, lhsT=selt[:, t * P:(t + 1) * P],
                                 rhs=stw_bf[:, w * 2 * NH + NH:(w + 1) * 2 * NH],
                                 start=False, stop=False)
                nc.tensor.matmul(seg, lhsT=ident[:],
                                 rhs=g3f[:, t, 64:64 + NH],
                                 start=False, stop=True)
            z = ps_st[:, 0:t_w * NH]
            # leaky(z) = max(z, 0.2*z); zmax tracks post-leaky values
            zs = sb.tile([P, t_max * NH], F32, tag="zs")
            nc.scalar.activation(zs[:, 0:t_w * NH], z, ACT.Copy, scale=LEAKY)
            zl = sb.tile([P, t_max * NH], F32, tag="zl")
            nc.vector.tensor_tensor(zl[:, 0:t_w * NH], z, zs[:, 0:t_w * NH],
                                    OP.max)
            nc.vector.tensor_tensor(zmax[:, 0:t_w * NH], zmax[:, 0:t_w * NH],
                                    zl[:, 0:t_w * NH], OP.max)
            ex = sb.tile([P, t_max * NH], BF16, tag="ex")
            nc.scalar.activation(ex[:, 0:t_w * NH], zl[:, 0:t_w * NH],
                                 ACT.Exp, bias=bias_mshift[:])
            wgt = sbg.tile([P, t_max * NHF], BF16, tag="wgt", bufs=3)
            ex3 = ex[:, 0:t_w * NH].rearrange("p (t h) -> p t h", h=NH)
            nc.vector.tensor_tensor(
                wgt[:, 0:t_w * NHF].rearrange("p (t h f) -> p t h f", h=NH,
                                              f=FOUT),
                g3[:, :, 0:NHF].rearrange("p t (h f) -> p t h f", f=FOUT),
                ex3[:, :, :, None].to_broadcast([P, t_w, NH, FOUT]),
                OP.mult)

            ps_w = psE.tile([P, P], F32, tag="ps_w")
            ps_d = psE.tile([NH, P], F32, tag="ps_d", bufs=1)
            for t in range(t_w):
                nc.tensor.matmul(ps_w[:], lhsT=wgt[:, t * NHF:(t + 1) * NHF],
                                 rhs=sel[:, t * P:(t + 1) * P],
                                 start=(t == 0), stop=(t == t_w - 1))
                nc.tensor.matmul(ps_d[:], lhsT=ex[:, t * NH:(t + 1) * NH],
                                 rhs=sel[:, t * P:(t + 1) * P],
                                 start=(t == 0), stop=(t == t_w - 1))
            nc.scalar.copy(acc_wt[:, w * P:(w + 1) * P], ps_w[:])
            nc.scalar.copy(acc_d[:, w * P:(w + 1) * P], ps_d[:])

        # --- global max + epsilon scalar ---
        zm1 = sb.tile([P, 1], F32, tag="zm1")
        nc.vector.tensor_reduce(zm1[:], zmax[:], axis=AX.X, op=OP.max)
        import bass_isa
        zm0 = sb.tile([P, 1], F32, tag="zm0")
        nc.gpsimd.partition_all_reduce(zm0[:], zm1[:], channels=P,
                                       reduce_op=bass_isa.ReduceOp.max)
        cc_in = dram.tile([1, 1], F32)
        cc_out = dram.tile([1, n_cores], F32)
        nc.sync.dma_start(cc_in[:], zm0[:1, :])
        nc.gpsimd.collective_compute(
            "AllGather", OP.bypass, replica_groups=[list(range(n_cores))],
            ins=[cc_in.opt()], outs=[cc_out.opt()])
        zg8 = sb.tile([1, n_cores], F32, tag="zg8")
        nc.sync.dma_start(zg8[:], cc_out[:])
        zg = sb.tile([1, 1], F32, tag="zg")
        nc.vector.tensor_reduce(zg[:], zg8[:], axis=AX.X, op=OP.max)
        # ceps = 1e-16 * exp(M - SHIFT) = exp(M - SHIFT + ln(1e-16))
        ce = sb.tile([1, 1], F32, tag="ce")
        nc.scalar.activation(ce[:], zg[:], ACT.Exp, bias=bias_meps[:1])
        ceps = const.tile([P, 1], F32)
        nc.gpsimd.partition_broadcast(ceps[:], ce[:])

        # bias folded into resident xloc (xlb = x_local + bias)
        FB = 4
        for g4 in range(0, nw, FB):
            gn = min(FB, nw - g4)
            r0 = g4 * P
            rows = min(FB * P, npc - r0)
            nc.vector.tensor_tensor(
                xloc_sb[:, r0:r0 + gn * NHF].rearrange(
                    "p (c f) -> p c f", f=NHF),
                xloc_sb[:, r0:r0 + gn * NHF].rearrange(
                    "p (c f) -> p c f", f=NHF),
                sb_b[:, None, :].to_broadcast([P, gn, NHF]), OP.add)

        # --- phase F: all denominators -> one reciprocal ---
        ps_dall = psE.tile([P, nw * NH], F32, tag="ps_d", bufs=1)
        for w in range(nw):
            nc.tensor.transpose(ps_dall[:, w * NH:(w + 1) * NH],
                                acc_d[:, w * P:(w + 1) * P], ident[:NH, :NH])
        rec_all = const.tile([P, nw * NH], F32)
        nc.vector.tensor_tensor(rec_all[:], ps_dall[:, 0:nw * NH],
                                ceps[:, :1].to_broadcast([P, nw * NH]),
                                OP.add)
        nc.vector.reciprocal(rec_all[:], rec_all[:])

        # --- phase F (8 windows per op batch) ---
        for g4 in range(0, nw, FB):
            gn = min(FB, nw - g4)
            r0 = g4 * P
            rows = min(FB * P, npc - r0)
            ps_w2 = psE.tile([P, FB * P], F32, tag="ps_wf", bufs=3)
            for ci in range(gn):
                w = g4 + ci
                nc.tensor.transpose(ps_w2[:, ci * P:(ci + 1) * P],
                                    acc_wt[:, w * P:(w + 1) * P], ident[:])
            o1 = sb.tile([P, FB * NHF], F32, tag="o1", bufs=2)
            r3 = rec_all[:, g4 * NH:(g4 + gn) * NH].rearrange(
                "p (c h) -> p c h", h=NH)
            nc.vector.tensor_tensor(
                o1[:, 0:gn * NHF].rearrange("p (c h f) -> p c h f", h=NH,
                                            f=FOUT),
                ps_w2[:, 0:gn * P].rearrange("p (c h f) -> p c h f", h=NH,
                                             f=FOUT),
                r3[:, :, :, None].to_broadcast([P, gn, NH, FOUT]),
                OP.mult)
            nc.gpsimd.tensor_tensor(o1[:, 0:gn * NHF], o1[:, 0:gn * NHF],
                                    xloc_sb[:, r0:r0 + gn * NHF], OP.add)
            # elu: exp(min(x,0)) == min(exp(x),1), so exp runs off o1 directly
            en4 = sb.tile([P, FB * NHF], F32, tag="en4", bufs=2)
            nc.scalar.activation(en4[:, 0:gn * NHF], o1[:, 0:gn * NHF],
                                 ACT.Exp, bias=bias_zero[:])
            nc.vector.tensor_scalar(en4[:, 0:gn * NHF], en4[:, 0:gn * NHF],
                                    1.0, -1.0, OP.min, op1=OP.add)
            pos = sb.tile([P, FB * NHF], F32, tag="pos", bufs=2)
            nc.scalar.activation(pos[:, 0:gn * NHF], o1[:, 0:gn * NHF],
                                 ACT.Relu)
            nc.vector.tensor_tensor(en4[:, 0:gn * NHF], en4[:, 0:gn * NHF],
                                    pos[:, 0:gn * NHF], OP.add)
            e3 = en4[:, 0:gn * NHF].rearrange("p (c f) -> p c f", f=NHF)
            full = rows // P
            rem = rows - full * P
            if full > 0:
                nc.sync.dma_start(
                    out[r0:r0 + full * P, :].rearrange(
                        "(c p) f -> p c f", p=P),
                    e3[:, 0:full, :])
            if rem > 0:
                nc.sync.dma_start(out[r0 + full * P:r0 + rows, :],
                                  e3[:rem, full, :])

    nc.compile()
    return nc


def _make_inputs(x, edge_index, w_mat, a_src, a_trg, bias, n_cores):
    n_nodes = x.shape[0]
    npc = n_nodes // n_cores
    t_as, t_bs, n_as, n_bs, gi, selt, sele = _prepare_edges(
        edge_index, n_nodes, n_cores)
    amat = np.zeros((NHF, 2 * NH), np.float32)
    for h in range(NH):
        amat[h * FOUT:(h + 1) * FOUT, h] = a_src[h]
        amat[h * FOUT:(h + 1) * FOUT, NH + h] = a_trg[h]
    x = np.ascontiguousarray(x, dtype=np.float32)
    in_maps = []
    for c in range(n_cores):
        in_maps.append({
            "x": x,
            "xloc": np.ascontiguousarray(x[c * npc:(c + 1) * npc]),
            "W": np.ascontiguousarray(w_mat, dtype=np.float32),
            "amat": amat,
            "bias": np.ascontiguousarray(bias, dtype=np.float32).reshape(1, NHF),
            "gi": np.ascontiguousarray(gi[c]),
            "selt": np.ascontiguousarray(selt[c]),
            "sele": np.ascontiguousarray(sele[c]),
        })
    return t_as, t_bs, n_as, n_bs, in_maps


def kernel(x, edge_index, W, a_src, a_trg, bias, _trace=False):
    from concourse.bass_utils import run_bass_kernel_spmd

    n_cores = 8
    x = np.asarray(x)
    n_nodes = x.shape[0]
    t_as, t_bs, n_as, n_bs, in_maps = _make_inputs(
        np.asarray(x), np.asarray(edge_index), np.asarray(W),
        np.asarray(a_src), np.asarray(a_trg), np.asarray(bias), n_cores)
    nc = build_bass(n_nodes, n_cores, t_as, t_bs, n_as, n_bs)
    res = run_bass_kernel_spmd(nc, in_maps, core_ids=list(range(n_cores)),
                               trace=_trace)
    out = np.concatenate([res.results[c]["out"] for c in range(n_cores)], axis=0)
    if _trace:
        kernel.last_results = res
    return out.astype(np.float32)
